# revision 23
# baseline (speedup 1.0000x reference)
"""Dynamic per-sample 3D Gaussian blur on 8 NeuronCores.

Sharding: pure data parallelism over (batch=4) x (channel=2) -> 8 cores,
one [160,160,160] volume per core. Per core the separable blur runs as
three banded-matmul passes on the TensorEngine with "zigzag" chunking:
each pass chunks so that every psum->SBUF evacuation writes a fully
CONTIGUOUS destination (strided SBUF writes cost ~4 cyc/elem on DVE/ACT)
and pass 1 consumes input slabs as they stream in from HBM:

  pass1 (conv d): x0[d', (h, w)] chunk h  -> psum[w, d] -> x1[w, (h, d)]
  pass2 (conv w): x1[w', (h, d)] chunk d  -> psum[h, w] -> x2[h, (d, w)]
  pass3 (conv h): x2[h', (d, w)] linear 128-chunks -> psum[(d,w), h] -> HBM

Pass-1/3 stationaries are contiguous 128/32-col slices (FWL weight
loads); pass-2 stationaries are 160-elem-strided h-columns at fixed d
(strided weight loads measured cheap; strided engine copies are not).
Input main rows stream via sync HW-DGE into f32 staging and are cast
f32->bf16 on VectorE (2x mode); the 32 tail rows go via GpSimd SW-DGE
DMAs that cast in the DMA datapath. The 160-long conv axis is split
128+32 on the contract dim; tail tensors share one [128, S] tile at
partition ranges [0:32)/[32:64)/[64:96) (matching gb(p) base
partitions). PSUM pa tiles batch 9 chunks (3 banks x 2 bufs) with
per-bank VectorE evacuation; pb (out-rows 128:160) tiles are 1 bank x 2
bufs, issued first in each group so their ScalarE copy is off the PE
critical path. Output is bf16 as y[p, k, h] (row k*128+p = d*160+w), so
each partition writes 2.88KB-contiguous DMA lines, alternating the
sync/gpsimd rings; host upcasts and permutes. Gaussians are computed on
host in fp32 exactly as the reference.
"""

from contextlib import ExitStack

import numpy as np
import ml_dtypes

import concourse.bass as bass
import concourse.tile as tile
from concourse import bacc, mybir
from concourse.bass_utils import run_bass_kernel_spmd

N = 160            # cube edge
S = N * N          # 25600 spatial positions per pass
NB = 13            # gaussian window
HALF = 6
A_N = 134          # main-block out-cols [0, 134): from contract rows [0,128)
B_C0 = 122         # tail-block out-cols [122, 160): from contract rows [128,160)
B_N = N - B_C0     # 38
GB_C0 = 3 * A_N    # col offset of the gb blocks in the packed G tile
G_COLS = 3 * A_N + B_N  # 440
EPS = 1e-7
GRP = 9            # psum chunks per pa tile (3 banks: 3*160 f32 per bank)
BANK = 512         # psum bank capacity in f32
PBG = 3            # psum chunks per bank / per pb tile
NG = (N + GRP - 1) // GRP   # 18 conv groups per pass
NK = S // 128               # 200 pass-3 chunks
NKG = (NK + GRP - 1) // GRP # 23 pass-3 groups
LA = 2             # input-slab lookahead (groups)
# pass-1 group schedule (uniform GRP-chunk groups)
P1S = [(9 * k, min(9, N - 9 * k)) for k in range(18)]
TV_C0 = G_COLS          # Tv1 cols [440, 542): T_h[0:128, 0:102]
TV_N1 = 102             # out rows 0:102 from contract h' 0:108
TV_C1 = TV_C0 + TV_N1   # Tv2 cols [542, 600): out rows 102:160
TV_N2 = N - TV_N1       # 58; contract h' 96:160 (rows permuted, see _gpack)
G_COLS2 = TV_C1 + TV_N2  # 600
NC3 = S // BANK         # 50 pass-3 chunks of 512 (d, w) columns
P3G = 3                 # pass-3 chunks per psum tile

BF16 = ml_dtypes.bfloat16
F32 = mybir.dt.float32
BF = mybir.dt.bfloat16

_PROGRAM = None


def _gaussian_1d(sigma):
    # fp32 replica of reference._gaussian_1d for a single sigma
    loc = (np.arange(NB, dtype=np.float32) - np.float32((NB - 1) / 2.0))
    s = np.float32(sigma)
    g = np.exp(-(loc * loc) / (2.0 * s * s + np.float32(EPS))
               - np.log(np.sqrt(np.float32(2.0 * np.pi)) * s + np.float32(EPS)))
    g = g.astype(np.float32)
    return g / g.sum(dtype=np.float32)


def _band(g):
    # T[r, c] = g[r - c + HALF] on the band, zero elsewhere ('SAME' zero pad)
    t = np.zeros((N, N), np.float32)
    for k in range(NB):
        off = k - HALF  # r = c + off
        c0 = max(0, -off)
        c1 = min(N, N - off)
        idx = np.arange(c0, c1)
        t[idx + off, idx] = g[k]
    return t


def _gpack(sigmas):
    """[128, G_COLS] bf16 for conv-axis sigmas (pass order d, w, h):
    cols [p*134,(p+1)*134) = T_p[0:128, 0:134];
    cols [402:440) partitions [32p, 32p+32) = T_p[128:160, 122:160)."""
    out = np.zeros((128, G_COLS2), np.float32)
    for p in range(3):
        t = _band(_gaussian_1d(sigmas[p]))
        out[:, p * A_N:(p + 1) * A_N] = t[0:128, 0:A_N]
        out[32 * p:32 * (p + 1), GB_C0:G_COLS] = t[128:N, B_C0:N]
    # pass-3 T-stationary blocks (t is T_h after the loop):
    # Tv1 [h' 0:128, h_out 0:102]; Tv2 [64:128, 58]: partition rows are
    # h' 128:160 then 96:128, matching tails[64:96] (x2 tail) and
    # tails[96:128] (x2 halo copy) so lhsT/rhs partitions pair up.
    out[0:128, TV_C0:TV_C1] = t[0:128, 0:TV_N1]
    out[64:96, TV_C1:G_COLS2] = t[128:160, TV_N1:N]
    out[96:128, TV_C1:G_COLS2] = t[96:128, TV_N1:N]
    return out.astype(BF16)


def _off(j):
    return (j // PBG) * BANK + (j % PBG) * N


def _build_kernel(ctx, tc, x_in, g_in, y_out):
    nc = tc.nc

    gpool = ctx.enter_context(tc.tile_pool(name="g", bufs=1))
    big = ctx.enter_context(tc.tile_pool(name="big", bufs=1))
    stin = ctx.enter_context(tc.tile_pool(name="stin", bufs=2))
    stout = ctx.enter_context(tc.tile_pool(name="stout", bufs=3))
    ps1 = ctx.enter_context(tc.tile_pool(name="ps1", bufs=1, space="PSUM"))
    ps2 = ctx.enter_context(tc.tile_pool(name="ps2", bufs=5, space="PSUM"))

    gtile = gpool.tile([128, G_COLS2], BF)
    nc.sync.dma_start(gtile[:], g_in)

    def ga(p):  # [128, 134] base partition 0
        return gtile[:, (p - 1) * A_N:p * A_N]

    def gb(p):  # [32, 38] base partition 32*(p-1)
        return gtile[32 * (p - 1):32 * p, GB_C0:G_COLS]

    # persistent volumes; tails: [0:32) x0 (d' 128:160), [32:64) x1
    # (w' 128:160), [64:96) x2 (h' 128:160)
    x0 = big.tile([128, S], BF, tag="sA")      # [d', (h, w)]
    x1 = big.tile([128, S], BF, tag="sB")      # [w, (h, d)]
    tails = big.tile([128, S], BF, tag="sT")

    # ---- input: HW-DGE DMAs to f32 staging (queued upfront, self-paced
    # by staging WAR deps), then f32->bf16 casts on DVE/ACT per group
    slabs = {}

    def issue_input_dma(g):
        if g >= len(P1S) or g in slabs:
            return
        a0, glen = P1S[g]
        L = glen * N
        sa = stin.tile([128, GRP * N], F32, tag="ia", name=f"ia{g}",
                       uniquify=True)
        nc.sync.dma_start(sa[0:128, 0:L], x_in[0:128, a0 * N:a0 * N + L])
        # tail rows go via GpSimd SW-DGE which casts f32->bf16 in flight
        nc.gpsimd.dma_start(tails[0:32, a0 * N:a0 * N + L],
                            x_in[128:160, a0 * N:a0 * N + L])
        slabs[g] = sa

    cast_done = set()

    def issue_cast(g):
        if g >= len(P1S) or g in cast_done:
            return
        cast_done.add(g)
        a0, glen = P1S[g]
        L = glen * N
        nc.vector.tensor_copy(x0[:, a0 * N:a0 * N + L], slabs[g][0:128, 0:L])

    # copy-engine round robin
    def eng(i):
        return nc.vector.tensor_copy if i % 2 == 0 else nc.scalar.copy

    def conv_group(p, get_main, get_tail, dstv, dstt, pb2, a0, glen):
        """One group (GRP chunks) of a rotation pass.
        get_main(g, j, c0, c1): stationary [contract 0:128, cols c0:c1]
        get_tail(g, j, c0, c1): stationary [contract 128:160 -> 32 rows]
        dstv / dstt: [*, N, N] chunk-major views; copy dst is contiguous
        pb2: psum partition base for the 32-row out-tail chunks."""
        if True:
            pa = ps1.tile([128, 3 * BANK], F32, tag="pa", name=f"pa{p}_{a0}",
                          uniquify=True)

            def pb_tile(q0):
                # one pb psum tile (<=3 chunks) + its ScalarE evacuation;
                # issued around the pa waves so the copy is off the PE
                # critical path (ps2 has only 2 buffers)
                qlen = min(PBG, glen - q0)
                pb = ps2.tile([128, BANK], F32, tag="pb",
                              name=f"pb{p}_{a0}_{q0}", uniquify=True)
                for j in range(q0, q0 + qlen):
                    nc.tensor.matmul(pb[pb2:pb2 + 32,
                                        (j - q0) * N:(j - q0) * N + A_N],
                                     get_main(a0 + j, 128, 160), ga(p),
                                     start=(j == q0), stop=False,
                                     skip_group_check=True)
                for j in range(q0, q0 + qlen):
                    o = (j - q0) * N
                    nc.tensor.matmul(pb[pb2:pb2 + 32, o + B_C0:o + N],
                                     get_tail(a0 + j, 128, 160), gb(p),
                                     start=False, stop=(j == q0 + qlen - 1),
                                     skip_group_check=True)
                src = pb[pb2:pb2 + 32, 0:qlen * N].rearrange(
                    "p (c d) -> p c d", d=N)
                nc.scalar.copy(dstt[:, a0 + q0:a0 + q0 + qlen, 0:N], src)

            pb_tile(0)
            for j in range(glen):
                o = _off(j)
                nc.tensor.matmul(pa[0:128, o:o + A_N], get_main(a0 + j, 0, 128),
                                 ga(p), start=(j % PBG == 0), stop=False,
                                 skip_group_check=True)
            for j in range(glen):
                o = _off(j)
                nc.tensor.matmul(pa[0:128, o + B_C0:o + N],
                                 get_tail(a0 + j, 0, 128), gb(p),
                                 start=False, stop=(j % PBG == PBG - 1 or
                                                    j == glen - 1),
                                 skip_group_check=True)
            for q0 in range(PBG, glen, PBG):
                pb_tile(q0)
            # per-bank pa evacuation on VectorE, contiguous dst
            for b in range((glen + PBG - 1) // PBG):
                qlen = min(PBG, glen - b * PBG)
                src = pa[0:128, b * BANK:b * BANK + qlen * N].rearrange(
                    "p (c d) -> p c d", d=N)
                dst = dstv[:, a0 + b * PBG:a0 + b * PBG + qlen, 0:N]
                nc.vector.tensor_copy(dst, src)

    # ---- pass 1: conv d; chunk h; consumes x0 slab-by-slab.
    for g in range(len(P1S)):
        issue_input_dma(g)
    for g in range(LA + 1):
        issue_cast(g)

    def p1_main(cj, c0, c1):
        return x0[:, cj * N + c0:cj * N + c1]

    def p1_tail(cj, c0, c1):
        return tails[0:32, cj * N + c0:cj * N + c1]

    x1v = x1[:].rearrange("p (h d) -> p h d", d=N)
    t1v = tails[32:64].rearrange("p (h d) -> p h d", d=N)
    for g, (a0, glen) in enumerate(P1S):
        issue_cast(g + LA + 1)
        conv_group(1, p1_main, p1_tail, x1v, t1v, 32, a0, glen)

    # x2 reuses x0's slot (x0 fully consumed by pass 1)
    x2 = big.tile([128, S], BF, tag="sA")      # [h, (d, w)]

    # ---- pass 2: conv w; chunk d; strided stationaries (h cols at
    # fixed d); psum [h, w] -> x2 [h, (d, w)] contiguous
    def p2_main(cj, c0, c1):
        return x1v[:, c0:c1, cj]

    def p2_tail(cj, c0, c1):
        return t1v[:, c0:c1, cj]

    x2v = x2[:].rearrange("p (d w) -> p d w", w=N)
    t2v = tails[64:96].rearrange("p (d w) -> p d w", w=N)
    # ---- pass 3: conv h; linear 128-row chunks of (d, w); to HBM bf16.
    # y_out is [128, 200*160]: partition p, cols (k, h) with chunk row
    # r = k*128 + p -> each partition writes 2.88KB-contiguous DMA lines.
    # Interleaved into pass 2: x2 cols fill in d-order, and p3 chunk c
    # only reads x2/tails cols [c*128, (c+1)*128).
    y3v = y_out.rearrange("p (k h) -> p k h", h=N)

    def p3_group(g):
        k0 = g * GRP
        glen = min(GRP, NK - k0)
        pa = ps1.tile([128, 3 * BANK], F32, tag="pa", name=f"p3_{g}",
                      uniquify=True)
        for j in range(glen):
            o = _off(j)
            c = (k0 + j) * 128
            nc.tensor.matmul(pa[0:128, o:o + A_N], x2[:, c:c + 128], ga(3),
                             start=(j % PBG == 0), stop=False,
                             skip_group_check=True)
        for j in range(glen):
            o = _off(j)
            c = (k0 + j) * 128
            nc.tensor.matmul(pa[0:128, o + B_C0:o + N],
                             tails[64:96, c:c + 128], gb(3),
                             start=False, stop=(j % PBG == PBG - 1 or
                                                j == glen - 1),
                             skip_group_check=True)
        st = stout.tile([128, GRP * N], BF, tag="so", name=f"st{g}",
                        uniquify=True)
        for b in range((glen + PBG - 1) // PBG):
            qlen = min(PBG, glen - b * PBG)
            eng(g + b)(st[:, b * PBG * N:b * PBG * N + qlen * N],
                       pa[0:128, b * BANK:b * BANK + qlen * N])
        dma_eng = nc.sync if g % 2 == 0 else nc.gpsimd
        dma_eng.dma_start(y3v[0:128, k0:k0 + glen, :],
                          st[:].rearrange("p (k h) -> p k h", h=N)[:, 0:glen, :])

    for g in range(NG):
        conv_group(2, p2_main, p2_tail, x2v, t2v, 64, g * GRP,
                   min(GRP, N - g * GRP))
    for g3 in range(NKG):
        p3_group(g3)


def _build_program():
    global _PROGRAM
    if _PROGRAM is not None:
        return _PROGRAM
    nc = bacc.Bacc("TRN2", target_bir_lowering=False, debug=False,
                   num_devices=8)
    x_in = nc.dram_tensor("x_in", [N, S], F32, kind="ExternalInput").ap()
    g_in = nc.dram_tensor("g_in", [128, G_COLS2], BF, kind="ExternalInput").ap()
    y_out = nc.dram_tensor("y_out", [128, NK * N], BF, kind="ExternalOutput").ap()
    with tile.TileContext(nc) as tc, ExitStack() as ctx:
        _build_kernel(ctx, tc, x_in, g_in, y_out)
    nc.compile()
    _PROGRAM = nc
    return nc


def _run(image, sigma, **spmd_kwargs):
    nc = _build_program()
    B, _, _, _, C = image.shape
    in_maps = []
    for core in range(8):
        b, c = divmod(core, C)
        vol = np.ascontiguousarray(image[b, :, :, :, c]).reshape(N, S)
        # pass conv-axis order is (d, w, h) -> sigma rows [0, 2, 1]
        in_maps.append({"x_in": vol,
                        "g_in": _gpack(sigma[b][[0, 2, 1]])})
    res = run_bass_kernel_spmd(nc, in_maps, list(range(8)), **spmd_kwargs)
    out = np.empty((B, N, N, N, C), np.float32)
    for core in range(8):
        b, c = divmod(core, C)
        y = np.asarray(res.results[core]["y_out"]).astype(np.float32)
        # y[p, k, h]: row r = k*128 + p = d*160 + w  ->  (d, h, w)
        y = y.reshape(128, NK, N).transpose(1, 0, 2).reshape(N, N, N)
        out[b, :, :, :, c] = y.transpose(0, 2, 1)
    return out, res


def kernel(image, sigma):
    image = np.asarray(image, dtype=np.float32)
    sigma = np.asarray(sigma, dtype=np.float32)
    out, _ = _run(image, sigma)
    return out


# revision 24
# speedup vs baseline: 1.3595x; 1.3595x over previous
"""Dynamic per-sample 3D Gaussian blur on 8 NeuronCores.

Sharding: pure data parallelism over (batch=4) x (channel=2) -> 8 cores,
one [160,160,160] volume per core. Per core the separable blur runs as
three banded-matmul passes on the TensorEngine with "zigzag" chunking:
each pass chunks so that every psum->SBUF evacuation writes a fully
CONTIGUOUS destination (strided SBUF writes cost ~4 cyc/elem on DVE/ACT)
and pass 1 consumes input slabs as they stream in from HBM:

  pass1 (conv d): x0[d', (h, w)] chunk h  -> psum[w, d] -> x1[w, (h, d)]
  pass2 (conv w): x1[w', (h, d)] chunk d  -> psum[h, w] -> x2[h, (d, w)]
  pass3 (conv h): x2[h', (d, w)] linear 128-chunks -> psum[(d,w), h] -> HBM

Pass-1/3 stationaries are contiguous 128/32-col slices (FWL weight
loads); pass-2 stationaries are 160-elem-strided h-columns at fixed d
(strided weight loads measured cheap; strided engine copies are not).
Input main rows stream via sync HW-DGE into f32 staging and are cast
f32->bf16 on VectorE (2x mode); the 32 tail rows go via GpSimd SW-DGE
DMAs that cast in the DMA datapath. The 160-long conv axis is split
128+32 on the contract dim; tail tensors share one [128, S] tile at
partition ranges [0:32)/[32:64)/[64:96) (matching gb(p) base
partitions). PSUM pa tiles batch 9 chunks (3 banks x 2 bufs) with
per-bank VectorE evacuation; pb (out-rows 128:160) tiles are 1 bank x 2
bufs, issued first in each group so their ScalarE copy is off the PE
critical path. Output is bf16 as y[p, k, h] (row k*128+p = d*160+w), so
each partition writes 2.88KB-contiguous DMA lines, alternating the
sync/gpsimd rings; host upcasts and permutes. Gaussians are computed on
host in fp32 exactly as the reference.
"""

from contextlib import ExitStack

import numpy as np
import ml_dtypes

import concourse.bass as bass
import concourse.tile as tile
from concourse import bacc, mybir
from concourse.bass_utils import run_bass_kernel_spmd

N = 160            # cube edge
S = N * N          # 25600 spatial positions per pass
NB = 13            # gaussian window
HALF = 6
A_N = 134          # main-block out-cols [0, 134): from contract rows [0,128)
B_C0 = 122         # tail-block out-cols [122, 160): from contract rows [128,160)
B_N = N - B_C0     # 38
GB_C0 = 3 * A_N    # col offset of the gb blocks in the packed G tile
G_COLS = 3 * A_N + B_N  # 440
EPS = 1e-7
GRP = 9            # psum chunks per pa tile (3 banks: 3*160 f32 per bank)
BANK = 512         # psum bank capacity in f32
PBG = 3            # psum chunks per bank / per pb tile
NG = (N + GRP - 1) // GRP   # 18 conv groups per pass
NK = S // 128               # 200 pass-3 chunks
NKG = (NK + GRP - 1) // GRP # 23 pass-3 groups
LA = 2             # input-slab lookahead (groups)
# pass-1 group schedule (uniform GRP-chunk groups)
P1S = [(9 * k, min(9, N - 9 * k)) for k in range(18)]
TV_C0 = G_COLS          # Tv1 cols [440, 542): T_h[0:128, 0:102]
TV_N1 = 102             # out rows 0:102 from contract h' 0:108
TV_C1 = TV_C0 + TV_N1   # Tv2 cols [542, 600): out rows 102:160
TV_N2 = N - TV_N1       # 58; contract h' 96:160 (rows permuted, see _gpack)
G_COLS2 = TV_C1 + TV_N2  # 600
NC3 = S // BANK         # 50 pass-3 chunks of 512 (d, w) columns
P3G = 3                 # pass-3 chunks per psum tile

BF16 = ml_dtypes.bfloat16
F32 = mybir.dt.float32
BF = mybir.dt.bfloat16

_PROGRAM = None


def _gaussian_1d(sigma):
    # fp32 replica of reference._gaussian_1d for a single sigma
    loc = (np.arange(NB, dtype=np.float32) - np.float32((NB - 1) / 2.0))
    s = np.float32(sigma)
    g = np.exp(-(loc * loc) / (2.0 * s * s + np.float32(EPS))
               - np.log(np.sqrt(np.float32(2.0 * np.pi)) * s + np.float32(EPS)))
    g = g.astype(np.float32)
    return g / g.sum(dtype=np.float32)


def _band(g):
    # T[r, c] = g[r - c + HALF] on the band, zero elsewhere ('SAME' zero pad)
    t = np.zeros((N, N), np.float32)
    for k in range(NB):
        off = k - HALF  # r = c + off
        c0 = max(0, -off)
        c1 = min(N, N - off)
        idx = np.arange(c0, c1)
        t[idx + off, idx] = g[k]
    return t


def _gpack(sigmas):
    """[128, G_COLS] bf16 for conv-axis sigmas (pass order d, w, h):
    cols [p*134,(p+1)*134) = T_p[0:128, 0:134];
    cols [402:440) partitions [32p, 32p+32) = T_p[128:160, 122:160)."""
    out = np.zeros((128, G_COLS2), np.float32)
    for p in range(3):
        t = _band(_gaussian_1d(sigmas[p]))
        out[:, p * A_N:(p + 1) * A_N] = t[0:128, 0:A_N]
        out[32 * p:32 * (p + 1), GB_C0:G_COLS] = t[128:N, B_C0:N]
    # pass-3 T-stationary blocks (t is T_h after the loop):
    # Tv1 [h' 0:128, h_out 0:102]; Tv2 [64:128, 58]: partition rows are
    # h' 128:160 then 96:128, matching tails[64:96] (x2 tail) and
    # tails[96:128] (x2 halo copy) so lhsT/rhs partitions pair up.
    out[0:128, TV_C0:TV_C1] = t[0:128, 0:TV_N1]
    out[64:96, TV_C1:G_COLS2] = t[128:160, TV_N1:N]
    out[96:128, TV_C1:G_COLS2] = t[96:128, TV_N1:N]
    return out.astype(BF16)


def _off(j):
    return (j // PBG) * BANK + (j % PBG) * N


def _build_kernel(ctx, tc, x_in, g_in, y_out):
    nc = tc.nc

    gpool = ctx.enter_context(tc.tile_pool(name="g", bufs=1))
    big = ctx.enter_context(tc.tile_pool(name="big", bufs=1))
    stin = ctx.enter_context(tc.tile_pool(name="stin", bufs=2))
    stout = ctx.enter_context(tc.tile_pool(name="stout", bufs=3))
    ps1 = ctx.enter_context(tc.tile_pool(name="ps1", bufs=2, space="PSUM"))
    ps2 = ctx.enter_context(tc.tile_pool(name="ps2", bufs=2, space="PSUM"))

    gtile = gpool.tile([128, G_COLS2], BF)
    nc.sync.dma_start(gtile[:], g_in)

    def ga(p):  # [128, 134] base partition 0
        return gtile[:, (p - 1) * A_N:p * A_N]

    def gb(p):  # [32, 38] base partition 32*(p-1)
        return gtile[32 * (p - 1):32 * p, GB_C0:G_COLS]

    # persistent volumes; tails: [0:32) x0 (d' 128:160), [32:64) x1
    # (w' 128:160), [64:96) x2 (h' 128:160)
    x0 = big.tile([128, S], BF, tag="sA")      # [d', (h, w)]
    x1 = big.tile([128, S], BF, tag="sB")      # [w, (h, d)]
    tails = big.tile([128, S], BF, tag="sT")

    # ---- input: HW-DGE DMAs to f32 staging (queued upfront, self-paced
    # by staging WAR deps), then f32->bf16 casts on DVE/ACT per group
    slabs = {}

    def issue_input_dma(g):
        if g >= len(P1S) or g in slabs:
            return
        a0, glen = P1S[g]
        L = glen * N
        sa = stin.tile([128, GRP * N], F32, tag="ia", name=f"ia{g}",
                       uniquify=True)
        nc.sync.dma_start(sa[0:128, 0:L], x_in[0:128, a0 * N:a0 * N + L])
        # tail rows go via GpSimd SW-DGE which casts f32->bf16 in flight
        nc.gpsimd.dma_start(tails[0:32, a0 * N:a0 * N + L],
                            x_in[128:160, a0 * N:a0 * N + L])
        slabs[g] = sa

    cast_done = set()

    def issue_cast(g):
        if g >= len(P1S) or g in cast_done:
            return
        cast_done.add(g)
        a0, glen = P1S[g]
        L = glen * N
        nc.vector.tensor_copy(x0[:, a0 * N:a0 * N + L], slabs[g][0:128, 0:L])

    # copy-engine round robin
    def eng(i):
        return nc.vector.tensor_copy if i % 2 == 0 else nc.scalar.copy

    def conv_group(p, get_main, get_tail, dstv, dstt, pb2, a0, glen):
        """One group (GRP chunks) of a rotation pass.
        get_main(g, j, c0, c1): stationary [contract 0:128, cols c0:c1]
        get_tail(g, j, c0, c1): stationary [contract 128:160 -> 32 rows]
        dstv / dstt: [*, N, N] chunk-major views; copy dst is contiguous
        pb2: psum partition base for the 32-row out-tail chunks."""
        if True:
            pa = ps1.tile([128, 3 * BANK], F32, tag="pa", name=f"pa{p}_{a0}",
                          uniquify=True)

            def pb_tile(q0):
                # one pb psum tile (<=3 chunks) + its ScalarE evacuation;
                # issued around the pa waves so the copy is off the PE
                # critical path (ps2 has only 2 buffers)
                qlen = min(PBG, glen - q0)
                pb = ps2.tile([128, BANK], F32, tag="pb",
                              name=f"pb{p}_{a0}_{q0}", uniquify=True)
                for j in range(q0, q0 + qlen):
                    nc.tensor.matmul(pb[pb2:pb2 + 32,
                                        (j - q0) * N:(j - q0) * N + A_N],
                                     get_main(a0 + j, 128, 160), ga(p),
                                     start=(j == q0), stop=False,
                                     skip_group_check=True)
                for j in range(q0, q0 + qlen):
                    o = (j - q0) * N
                    nc.tensor.matmul(pb[pb2:pb2 + 32, o + B_C0:o + N],
                                     get_tail(a0 + j, 128, 160), gb(p),
                                     start=False, stop=(j == q0 + qlen - 1),
                                     skip_group_check=True)
                src = pb[pb2:pb2 + 32, 0:qlen * N].rearrange(
                    "p (c d) -> p c d", d=N)
                nc.scalar.copy(dstt[:, a0 + q0:a0 + q0 + qlen, 0:N], src)

            pb_tile(0)
            for j in range(glen):
                o = _off(j)
                nc.tensor.matmul(pa[0:128, o:o + A_N], get_main(a0 + j, 0, 128),
                                 ga(p), start=(j % PBG == 0), stop=False,
                                 skip_group_check=True)
            for j in range(glen):
                o = _off(j)
                nc.tensor.matmul(pa[0:128, o + B_C0:o + N],
                                 get_tail(a0 + j, 0, 128), gb(p),
                                 start=False, stop=(j % PBG == PBG - 1 or
                                                    j == glen - 1),
                                 skip_group_check=True)
            for q0 in range(PBG, glen, PBG):
                pb_tile(q0)
            # per-bank pa evacuation on VectorE, contiguous dst
            for b in range((glen + PBG - 1) // PBG):
                qlen = min(PBG, glen - b * PBG)
                src = pa[0:128, b * BANK:b * BANK + qlen * N].rearrange(
                    "p (c d) -> p c d", d=N)
                dst = dstv[:, a0 + b * PBG:a0 + b * PBG + qlen, 0:N]
                nc.vector.tensor_copy(dst, src)

    # ---- pass 1: conv d; chunk h; consumes x0 slab-by-slab.
    for g in range(len(P1S)):
        issue_input_dma(g)
    for g in range(LA + 1):
        issue_cast(g)

    def p1_main(cj, c0, c1):
        return x0[:, cj * N + c0:cj * N + c1]

    def p1_tail(cj, c0, c1):
        return tails[0:32, cj * N + c0:cj * N + c1]

    x1v = x1[:].rearrange("p (h d) -> p h d", d=N)
    t1v = tails[32:64].rearrange("p (h d) -> p h d", d=N)
    for g, (a0, glen) in enumerate(P1S):
        issue_cast(g + LA + 1)
        conv_group(1, p1_main, p1_tail, x1v, t1v, 32, a0, glen)

    # x2 reuses x0's slot (x0 fully consumed by pass 1)
    x2 = big.tile([128, S], BF, tag="sA")      # [h, (d, w)]

    # ---- pass 2: conv w; chunk d; strided stationaries (h cols at
    # fixed d); psum [h, w] -> x2 [h, (d, w)] contiguous
    def p2_main(cj, c0, c1):
        return x1v[:, c0:c1, cj]

    def p2_tail(cj, c0, c1):
        return t1v[:, c0:c1, cj]

    x2v = x2[:].rearrange("p (d w) -> p d w", w=N)
    t2v = tails[64:96].rearrange("p (d w) -> p d w", w=N)
    # ---- pass 3: conv h; linear 128-row chunks of (d, w); to HBM bf16.
    # y_out is [128, 200*160]: partition p, cols (k, h) with chunk row
    # r = k*128 + p -> each partition writes 2.88KB-contiguous DMA lines.
    # Interleaved into pass 2: x2 cols fill in d-order, and p3 chunk c
    # only reads x2/tails cols [c*128, (c+1)*128).
    y3v = y_out.rearrange("p (k h) -> p k h", h=N)

    def p3_group(g):
        k0 = g * GRP
        glen = min(GRP, NK - k0)
        pa = ps1.tile([128, 3 * BANK], F32, tag="pa", name=f"p3_{g}",
                      uniquify=True)
        for j in range(glen):
            o = _off(j)
            c = (k0 + j) * 128
            nc.tensor.matmul(pa[0:128, o:o + A_N], x2[:, c:c + 128], ga(3),
                             start=(j % PBG == 0), stop=False,
                             skip_group_check=True)
        for j in range(glen):
            o = _off(j)
            c = (k0 + j) * 128
            nc.tensor.matmul(pa[0:128, o + B_C0:o + N],
                             tails[64:96, c:c + 128], gb(3),
                             start=False, stop=(j % PBG == PBG - 1 or
                                                j == glen - 1),
                             skip_group_check=True)
        st = stout.tile([128, GRP * N], BF, tag="so", name=f"st{g}",
                        uniquify=True)
        for b in range((glen + PBG - 1) // PBG):
            qlen = min(PBG, glen - b * PBG)
            eng(g + b)(st[:, b * PBG * N:b * PBG * N + qlen * N],
                       pa[0:128, b * BANK:b * BANK + qlen * N])
        dma_eng = nc.sync if g % 2 == 0 else nc.gpsimd
        dma_eng.dma_start(y3v[0:128, k0:k0 + glen, :],
                          st[:].rearrange("p (k h) -> p k h", h=N)[:, 0:glen, :])

    for g in range(NG):
        conv_group(2, p2_main, p2_tail, x2v, t2v, 64, g * GRP,
                   min(GRP, N - g * GRP))
    for g3 in range(NKG):
        p3_group(g3)


def _build_program():
    global _PROGRAM
    if _PROGRAM is not None:
        return _PROGRAM
    nc = bacc.Bacc("TRN2", target_bir_lowering=False, debug=False,
                   num_devices=8)
    x_in = nc.dram_tensor("x_in", [N, S], F32, kind="ExternalInput").ap()
    g_in = nc.dram_tensor("g_in", [128, G_COLS2], BF, kind="ExternalInput").ap()
    y_out = nc.dram_tensor("y_out", [128, NK * N], BF, kind="ExternalOutput").ap()
    with tile.TileContext(nc) as tc, ExitStack() as ctx:
        _build_kernel(ctx, tc, x_in, g_in, y_out)
    nc.compile()
    _PROGRAM = nc
    return nc


def _run(image, sigma, **spmd_kwargs):
    nc = _build_program()
    B, _, _, _, C = image.shape
    in_maps = []
    for core in range(8):
        b, c = divmod(core, C)
        vol = np.ascontiguousarray(image[b, :, :, :, c]).reshape(N, S)
        # pass conv-axis order is (d, w, h) -> sigma rows [0, 2, 1]
        in_maps.append({"x_in": vol,
                        "g_in": _gpack(sigma[b][[0, 2, 1]])})
    res = run_bass_kernel_spmd(nc, in_maps, list(range(8)), **spmd_kwargs)
    out = np.empty((B, N, N, N, C), np.float32)
    for core in range(8):
        b, c = divmod(core, C)
        y = np.asarray(res.results[core]["y_out"]).astype(np.float32)
        # y[p, k, h]: row r = k*128 + p = d*160 + w  ->  (d, h, w)
        y = y.reshape(128, NK, N).transpose(1, 0, 2).reshape(N, N, N)
        out[b, :, :, :, c] = y.transpose(0, 2, 1)
    return out, res


def kernel(image, sigma):
    image = np.asarray(image, dtype=np.float32)
    sigma = np.asarray(sigma, dtype=np.float32)
    out, _ = _run(image, sigma)
    return out
